# revision 20
# baseline (speedup 1.0000x reference)
"""Trainium2 Bass kernel for nn_Attention_15676630631260 (sparse_attention).

reference:
  q = x @ Wq.T + bq ; k = x @ Wk.T + bk ; v = x @ Wv.T + bv        (per batch)
  scores = sigmoid(q @ k.T / sqrt(P))                               [B,S,S]
  out[b,i,j,:] = tril(i,j) * scores[b,i,j] * v[b,j,:]               [B,S,S,P]

B=2, S=512, D=256, P=128.  Output is 256 MB; the causal mask zeroes the
j>i region.  run_bass_kernel_spmd pre-zeroes ExternalOutput buffers
(donated zero buffers under PJRT), so the kernel only writes the j<=i
region — at 128-column tile granularity per row: row i writes j-tiles
0..i//128 (the partial diagonal tile is zeroed exactly via a
host-supplied mask).

Sharding (8 cores, one NEFF, SPMD): core c -> batch b=c//4, quarter
k=c%4.  Rows are assigned as 16-row blocks paired (m, 31-m) so every
core's multiset of per-row written-tile-counts is {1,1,2,2,3,3,4,4} per
block pair -> identical instruction stream on every core, only input
data differs, and every core writes exactly 20 MB of the 32 MB shard.

Per-core device program:
  K^T[p,s], Q^T[p,i] with per-partition bias via K=1 matmul; V[s,p]
  tiles.  scores^T[j,i] = sigmoid((K^T_tile)^T @ Q^T / sqrt(P)) *
  mask.  Output rows are produced as [j_partition, (jt, i, p)] slabs:
  broadcast row-scaling of V by score columns, batched 8 rows per DVE
  tensor_tensor (stride-0 broadcast APs) with a slice of rows done as
  per-row activation-scale ops on ACT to balance the two engines; then
  batched HWDGE DMAs ([j, jt, (i p)] — 4 KB contiguous runs per
  partition) into the [j, i_local, p]-layout local output.
"""

import os
import sys

import numpy as np

for _p in ("/root/.axon_site/_ro/trn_rl_repo", "/opt/trn_rl_repo"):
    if _p not in sys.path and os.path.isdir(_p):
        sys.path.append(_p)

import concourse.bass as bass
import concourse.mybir as mybir
from concourse.tile import TileContext
from concourse import bass_utils

F32 = mybir.dt.float32
BF16 = mybir.dt.bfloat16
F32R = mybir.dt.float32r
B, S, D, P = 2, 512, 256, 128
NCORES = 8
GROUP = 8           # output rows per DMA group
NGROUPS = 128 // GROUP
INV_SQRT_P = float(1.0 / np.sqrt(np.float32(P)))
# Producer-engine schedule for the 40 (group, jt) group-tiles: D = one
# batched DVE tensor_tensor, A = 8 per-row ACT activation-scale ops,
# G = one batched GpSimd tensor_tensor.  Tuned for engine balance.
GT_PATTERN = "DDADDADDDADDADDD"


def _blocks16(k: int) -> list[int]:
    # 16-row blocks (32 per batch) for quarter k, ordered so written
    # j-tile count ti=m//8 ascends: [0,0,1,1,2,2,3,3]
    return [k, k + 4, k + 8, k + 12, 19 - k, 23 - k, 27 - k, 31 - k]


def _rows_sel(k: int) -> np.ndarray:
    return np.concatenate([np.arange(16 * m, 16 * m + 16) for m in _blocks16(k)])


def _build_nc() -> bass.Bass:
    nc = bass.Bass(trn_type="TRN2")

    xt = nc.dram_tensor("xt", [D, S], F32R, kind="ExternalInput")     # x[b].T
    xq = nc.dram_tensor("xq", [D, 128], F32R, kind="ExternalInput")   # x[b].T[:, rows]
    w3 = nc.dram_tensor("w3", [D, 3 * P], F32R, kind="ExternalInput")  # [Wq|Wk|Wv].T
    b3 = nc.dram_tensor("b3", [P, 3], F32, kind="ExternalInput")  # cols bq|bk|bv
    mk = nc.dram_tensor("mk", [4, 128, 128], F32, kind="ExternalInput")
    # local output layout [j, i_local, p]: per-DMA-partition runs are
    # (i,p)-contiguous (4 KB per 8-row group) instead of 512 B
    out = nc.dram_tensor("out", [S, 128, P], F32, kind="ExternalOutput")

    with TileContext(nc) as tc:
        with (
            tc.tile_pool(name="const", bufs=1) as cpool,
            tc.tile_pool(name="psA", bufs=1, space="PSUM") as psA,
            tc.tile_pool(name="psB", bufs=2, space="PSUM") as psB,
            tc.tile_pool(name="slab", bufs=3) as spool,
        ):
            # ---- input loads ----
            # Critical-path inputs (b3, Wq|Wk, xq, x-tile0, mask0, Wv) go on
            # the Sync HWDGE ring in need-order; the rest stream in parallel
            # on the ACT HWDGE ring.  Per-s-tile x/mask loads let tile-0
            # compute start long before all input bytes have landed.
            w3_r = w3.rearrange("(c p) m -> p c m", p=128)     # [128, 2, 384]
            xt_r = xt.rearrange("(c p) s -> p c s", p=128)     # [128, 2, 512]

            b_sb = cpool.tile([P, 3], F32, tag="b3")
            nc.sync.dma_start(b_sb[:], b3[:])
            wqk_sb = cpool.tile([128, 2 * 2 * P], F32R, tag="wqk")  # [c x (q|k)]
            nc.sync.dma_start(
                wqk_sb[:].rearrange("q (c m) -> q c m", c=2),
                w3_r[:, :, 0 : 2 * P],
            )
            xq_sb = cpool.tile([128, 2 * 128], F32R, tag="xq")
            nc.sync.dma_start(
                xq_sb[:].rearrange("q (c m) -> q c m", c=2),
                xq.rearrange("(c p) m -> p c m", p=128),
            )
            xt_t = []
            mk_t = []
            for jt in range(4):
                xtile = cpool.tile([128, 2 * 128], F32R, tag=f"xt{jt}", name=f"xt{jt}")
                xt_t.append(xtile)
                mtile = cpool.tile([128, 128], F32, tag=f"mk{jt}", name=f"mk{jt}")
                mk_t.append(mtile)

            def load_xt(jt, eng):
                eng.dma_start(
                    xt_t[jt][:].rearrange("q (c m) -> q c m", c=2),
                    xt_r[:, :, jt * 128 : (jt + 1) * 128],
                )

            def load_mk(jt, eng):
                eng.dma_start(mk_t[jt][:], mk[jt])

            load_xt(0, nc.sync)
            load_mk(0, nc.sync)
            wv_sb = cpool.tile([128, 2 * P], F32R, tag="wv")
            nc.sync.dma_start(
                wv_sb[:].rearrange("q (c m) -> q c m", c=2),
                w3_r[:, :, 2 * P : 3 * P],
            )
            for jt in range(1, 4):
                load_xt(jt, nc.scalar)
            for jt in range(1, 4):
                load_mk(jt, nc.scalar)

            bq_c = b_sb[:, 0:1]
            bk_c = b_sb[:, 1:2]
            bv_c = b_sb[:, 2:3]

            identity = cpool.tile([128, 128], F32, tag="ident")
            from concourse.masks import make_identity
            make_identity(nc, identity[:])

            def wqk(which, c):  # 0=q, 1=k
                return wqk_sb[:, (c * 2 + which) * P : (c * 2 + which + 1) * P]

            def wv(c):
                return wv_sb[:, c * P : (c + 1) * P]

            # Q^T [p, i] (fp32r single-pass)
            qt_ps = psA.tile([128, 128], F32, tag="qtps")
            nc.tensor.matmul(qt_ps[:], wqk(0, 0), xq_sb[:, 0:128], start=True, stop=False)
            nc.tensor.matmul(qt_ps[:], wqk(0, 1), xq_sb[:, 128:256], start=False, stop=True)
            qt_sb = cpool.tile([128, 128], F32R, tag="qt")
            nc.vector.tensor_scalar_add(qt_sb[:], qt_ps[:], bq_c)

            kt_t = [None] * 4
            v_t = [None] * 4
            stm_t = [None] * 4

            def make_tiles(jt: int):
                # K^T tile [p, s_jt]
                ktp = psB.tile([128, 128], F32, tag="proj", name=f"ktp{jt}")
                nc.tensor.matmul(ktp[:], wqk(1, 0), xt_t[jt][:, 0:128], start=True, stop=False)
                nc.tensor.matmul(ktp[:], wqk(1, 1), xt_t[jt][:, 128:256], start=False, stop=True)
                ktile = cpool.tile([128, 128], F32R, tag=f"kt{jt}", name=f"kt{jt}")
                nc.vector.tensor_scalar_add(ktile[:], ktp[:], bk_c)
                kt_t[jt] = ktile

                # scores [i, j_jt] -> sigmoid -> transpose -> mask
                sp = psB.tile([128, 128], F32, tag="sps", name=f"sps{jt}")
                nc.tensor.matmul(sp[:], qt_sb[:], ktile[:], start=True, stop=True)
                st = cpool.tile([128, 128], F32, tag=f"st{jt}", name=f"st{jt}")
                nc.scalar.activation(
                    st[:], sp[:], mybir.ActivationFunctionType.Sigmoid,
                    scale=INV_SQRT_P,
                )
                stp = psB.tile([128, 128], F32, tag="tp", name=f"stp{jt}")
                nc.tensor.transpose(stp[:], st[:], identity[:])
                stm = cpool.tile([128, 128], F32, tag=f"stm{jt}", name=f"stm{jt}")
                nc.vector.tensor_mul(stm[:], stp[:], mk_t[jt][:])
                stm_t[jt] = stm

                # V^T tile [p, s_jt] -> +bias -> transpose -> V [s_jt, p]
                vtp = psB.tile([128, 128], F32, tag="proj", name=f"vtp{jt}")
                nc.tensor.matmul(vtp[:], wv(0), xt_t[jt][:, 0:128], start=True, stop=False)
                nc.tensor.matmul(vtp[:], wv(1), xt_t[jt][:, 128:256], start=False, stop=True)
                vT = cpool.tile([128, 128], F32, tag=f"vT{jt}", name=f"vT{jt}")
                nc.vector.tensor_scalar_add(vT[:], vtp[:], bv_c)
                vp = psB.tile([128, 128], F32, tag="tp", name=f"vp{jt}")
                nc.tensor.transpose(vp[:], vT[:], identity[:])
                vt = cpool.tile([128, P], F32, tag=f"v{jt}", name=f"v{jt}")
                nc.vector.tensor_copy(vt[:], vp[:])
                v_t[jt] = vt

            # ---- output slab stage ----
            # groups of GROUP rows; class t = g//4 writes j-tiles 0..t.
            # slab free layout (jt, i, p); DMA [j_part, t, (i p)]: 4 KB runs.
            # Producers: greedy cost-balanced over DVE (batched TT), ACT
            # (per-row activation-scale), GpSimd (per-row tensor_scalar).
            out_r = out.rearrange("(t j) i p -> j t (i p)", j=128)  # [128,4,16384]
            busy = {"D": 0.0, "A": 0.0, "G": 0.0}
            cost = {"D": 1.25, "A": 3.9, "G": 3.6}

            def emit_group(g: int):
                L = g // (NGROUPS // 4) + 1
                slab = spool.tile(
                    [128, L * GROUP * 128], F32, tag=f"slab{L}", name=f"slab_g{g}"
                )
                for jt in range(L):
                    dst3 = slab[
                        :, jt * GROUP * 128 : (jt + 1) * GROUP * 128
                    ].rearrange("q (i p) -> q i p", i=GROUP)
                    eng = min(busy, key=lambda e: busy[e] + cost[e])
                    busy[eng] += cost[eng]
                    if eng == "D":
                        v3 = v_t[jt][:].unsqueeze(1).broadcast_to([128, GROUP, 128])
                        s3 = (
                            stm_t[jt][:, g * GROUP : (g + 1) * GROUP]
                            .unsqueeze(2)
                            .broadcast_to([128, GROUP, 128])
                        )
                        nc.vector.tensor_mul(dst3, v3, s3)
                    else:
                        for ii in range(GROUP):
                            li = g * GROUP + ii
                            if eng == "A":
                                nc.scalar.mul(
                                    dst3[:, ii, :],
                                    v_t[jt][:],
                                    mul=stm_t[jt][:, li : li + 1],
                                )
                            else:
                                nc.gpsimd.tensor_scalar_mul(
                                    dst3[:, ii, :],
                                    v_t[jt][:],
                                    stm_t[jt][:, li : li + 1],
                                )
                nc.sync.dma_start(
                    out_r[:, 0:L, GROUP * 128 * g : GROUP * 128 * (g + 1)],
                    slab[:].rearrange("q (t ip) -> q t ip", t=L),
                )

            make_tiles(0)
            for g in range(0, 4):      # class 0
                emit_group(g)
            for jt in range(1, 4):
                make_tiles(jt)
            for g in range(12, 16):    # class 3
                emit_group(g)
            for g in range(8, 12):     # class 2
                emit_group(g)
            for g in range(4, 8):      # class 1
                emit_group(g)

    _split_multi_waits(nc)
    return nc


def _split_multi_waits(nc):
    """This toolchain's walrus accepts at most one sync wait per
    instruction; split extras into single-wait NoOps just before the
    instruction on the same engine queue (waits are ANDed preconditions,
    executed in order on the engine's queue — semantically identical)."""
    for fn in nc.m.functions:
        for blk in fn.blocks:
            insts = blk.instructions
            i = 0
            while i < len(insts):
                inst = insts[i]
                si = getattr(inst, "sync_info", None)
                if si is not None and si.on_wait is not None and len(si.on_wait) > 1:
                    waits = list(si.on_wait)
                    nops = [
                        mybir.InstNoOp(
                            name=nc.get_next_instruction_name(),
                            engine=inst.engine,
                            sync_info=mybir.SyncInfo(on_wait=[w], on_update=[]),
                            bass_nofuse=True,
                        )
                        for w in waits[:-1]
                    ]
                    si.on_wait = [waits[-1]]
                    insts[i:i] = nops
                    i += len(nops)
                i += 1


_NC_CACHE = None


def _get_nc():
    global _NC_CACHE
    if _NC_CACHE is None:
        _NC_CACHE = _build_nc()
    return _NC_CACHE


def _in_maps(x_set, Wq, bq, Wk, bk, Wv, bv):
    w3 = np.ascontiguousarray(
        np.concatenate([Wq.T, Wk.T, Wv.T], axis=1)
    ).astype(np.float32, copy=False)
    b3 = np.ascontiguousarray(np.stack([bq, bk, bv], axis=1)).astype(np.float32, copy=False)
    xts = [
        np.ascontiguousarray(x_set[b].T).astype(np.float32, copy=False)
        for b in range(B)
    ]
    jj = np.arange(128)
    maps = []
    for c in range(NCORES):
        b, k = divmod(c, 4)
        rows = _rows_sel(k)
        mask = np.empty((4, 128, 128), np.float32)
        for jt in range(4):
            mask[jt] = ((jt * 128 + jj)[:, None] <= rows[None, :]).astype(np.float32)
        maps.append(
            {
                "xt": xts[b],
                "xq": np.ascontiguousarray(xts[b][:, rows]),
                "w3": w3,
                "b3": b3,
                "mk": mask,
            }
        )
    return maps


def run(x_set, Wq, bq, Wk, bk, Wv, bv, **spmd_kwargs):
    nc = _get_nc()
    in_maps = _in_maps(x_set, Wq, bq, Wk, bk, Wv, bv)
    res = bass_utils.run_bass_kernel_spmd(
        nc, in_maps, core_ids=list(range(NCORES)), **spmd_kwargs
    )
    full = np.zeros((B, S, S, P), np.float32)
    for c in range(NCORES):
        b, k = divmod(c, 4)
        # core output is [j, i_local, p] -> scatter as [i_local, j, p]
        full[b, _rows_sel(k)] = res.results[c]["out"].transpose(1, 0, 2)
    return full, res


def kernel(x_set, Wq, bq, Wk, bk, Wv, bv):
    full, _ = run(x_set, Wq, bq, Wk, bk, Wv, bv)
    return full


# revision 21
# speedup vs baseline: 2.2865x; 2.2865x over previous
"""Trainium2 Bass kernel for nn_Attention_15676630631260 (sparse_attention).

reference:
  q = x @ Wq.T + bq ; k = x @ Wk.T + bk ; v = x @ Wv.T + bv        (per batch)
  scores = sigmoid(q @ k.T / sqrt(P))                               [B,S,S]
  out[b,i,j,:] = tril(i,j) * scores[b,i,j] * v[b,j,:]               [B,S,S,P]

B=2, S=512, D=256, P=128.  Output is 256 MB; the causal mask zeroes the
j>i region.  run_bass_kernel_spmd pre-zeroes ExternalOutput buffers
(donated zero buffers under PJRT), so the kernel only writes the j<=i
region — at 128-column tile granularity per row: row i writes j-tiles
0..i//128 (the partial diagonal tile is zeroed exactly via a
host-supplied mask).

Sharding (8 cores, one NEFF, SPMD): core c -> batch b=c//4, quarter
k=c%4.  Rows are assigned as 16-row blocks paired (m, 31-m) so every
core's multiset of per-row written-tile-counts is {1,1,2,2,3,3,4,4} per
block pair -> identical instruction stream on every core, only input
data differs, and every core writes exactly 20 MB of the 32 MB shard.

Per-core device program:
  K^T[p,s], Q^T[p,i] with per-partition bias via K=1 matmul; V[s,p]
  tiles.  scores^T[j,i] = sigmoid((K^T_tile)^T @ Q^T / sqrt(P)) *
  mask.  Output rows are produced as [j_partition, (jt, i, p)] slabs:
  broadcast row-scaling of V by score columns, batched 8 rows per DVE
  tensor_tensor (stride-0 broadcast APs) with a slice of rows done as
  per-row activation-scale ops on ACT to balance the two engines; then
  batched HWDGE DMAs ([j, jt, (i p)] — 4 KB contiguous runs per
  partition) into the [j, i_local, p]-layout local output.
"""

import os
import sys

import numpy as np

for _p in ("/root/.axon_site/_ro/trn_rl_repo", "/opt/trn_rl_repo"):
    if _p not in sys.path and os.path.isdir(_p):
        sys.path.append(_p)

import concourse.bass as bass
import concourse.mybir as mybir
from concourse.tile import TileContext
from concourse import bass_utils

F32 = mybir.dt.float32
BF16 = mybir.dt.bfloat16
F32R = mybir.dt.float32r
B, S, D, P = 2, 512, 256, 128
NCORES = 8
GROUP = 8           # output rows per DMA group
NGROUPS = 128 // GROUP
INV_SQRT_P = float(1.0 / np.sqrt(np.float32(P)))
# Producer-engine schedule for the 40 (group, jt) group-tiles: D = one
# batched DVE tensor_tensor, A = 8 per-row ACT activation-scale ops,
# G = one batched GpSimd tensor_tensor.  Tuned for engine balance.
GT_PATTERN = "DDADDADDDADDADDD"


def _blocks16(k: int) -> list[int]:
    # 16-row blocks (32 per batch) for quarter k, ordered so written
    # j-tile count ti=m//8 ascends: [0,0,1,1,2,2,3,3]
    return [k, k + 4, k + 8, k + 12, 19 - k, 23 - k, 27 - k, 31 - k]


def _rows_sel(k: int) -> np.ndarray:
    return np.concatenate([np.arange(16 * m, 16 * m + 16) for m in _blocks16(k)])


def _build_nc() -> bass.Bass:
    nc = bass.Bass(trn_type="TRN2")

    xt = nc.dram_tensor("xt", [D, S], F32R, kind="ExternalInput")     # x[b].T
    xq = nc.dram_tensor("xq", [D, 128], F32R, kind="ExternalInput")   # x[b].T[:, rows]
    w3 = nc.dram_tensor("w3", [D, 3 * P], F32R, kind="ExternalInput")  # [Wq|Wk|Wv].T
    b3 = nc.dram_tensor("b3", [P, 3], F32, kind="ExternalInput")  # cols bq|bk|bv
    mk = nc.dram_tensor("mk", [4, 128, 128], F32, kind="ExternalInput")
    # local output layout [j, i_local, p]: per-DMA-partition runs are
    # (i,p)-contiguous (4 KB per 8-row group) instead of 512 B
    out = nc.dram_tensor("out", [S, 128, P], F32, kind="ExternalOutput")

    with TileContext(nc) as tc:
        with (
            tc.tile_pool(name="const", bufs=1) as cpool,
            tc.tile_pool(name="psA", bufs=1, space="PSUM") as psA,
            tc.tile_pool(name="psB", bufs=2, space="PSUM") as psB,
            tc.tile_pool(name="slab", bufs=3) as spool,
        ):
            # ---- input loads ----
            # Critical-path inputs (b3, Wq|Wk, xq, x-tile0, mask0, Wv) go on
            # the Sync HWDGE ring in need-order; the rest stream in parallel
            # on the ACT HWDGE ring.  Per-s-tile x/mask loads let tile-0
            # compute start long before all input bytes have landed.
            w3_r = w3.rearrange("(c p) m -> p c m", p=128)     # [128, 2, 384]
            xt_r = xt.rearrange("(c p) s -> p c s", p=128)     # [128, 2, 512]

            b_sb = cpool.tile([P, 3], F32, tag="b3")
            nc.sync.dma_start(b_sb[:], b3[:])
            wqk_sb = cpool.tile([128, 2 * 2 * P], F32R, tag="wqk")  # [c x (q|k)]
            nc.sync.dma_start(
                wqk_sb[:].rearrange("q (c m) -> q c m", c=2),
                w3_r[:, :, 0 : 2 * P],
            )
            xq_sb = cpool.tile([128, 2 * 128], F32R, tag="xq")
            nc.sync.dma_start(
                xq_sb[:].rearrange("q (c m) -> q c m", c=2),
                xq.rearrange("(c p) m -> p c m", p=128),
            )
            xt_t = []
            mk_t = []
            for jt in range(4):
                xtile = cpool.tile([128, 2 * 128], F32R, tag=f"xt{jt}", name=f"xt{jt}")
                xt_t.append(xtile)
                mtile = cpool.tile([128, 128], F32, tag=f"mk{jt}", name=f"mk{jt}")
                mk_t.append(mtile)

            def load_xt(jt, eng):
                eng.dma_start(
                    xt_t[jt][:].rearrange("q (c m) -> q c m", c=2),
                    xt_r[:, :, jt * 128 : (jt + 1) * 128],
                )

            def load_mk(jt, eng):
                eng.dma_start(mk_t[jt][:], mk[jt])

            load_xt(0, nc.sync)
            load_mk(0, nc.sync)
            wv_sb = cpool.tile([128, 2 * P], F32R, tag="wv")
            nc.sync.dma_start(
                wv_sb[:].rearrange("q (c m) -> q c m", c=2),
                w3_r[:, :, 2 * P : 3 * P],
            )
            for jt in range(1, 4):
                load_xt(jt, nc.scalar)
            for jt in range(1, 4):
                load_mk(jt, nc.scalar)

            bq_c = b_sb[:, 0:1]
            bk_c = b_sb[:, 1:2]
            bv_c = b_sb[:, 2:3]

            identity = cpool.tile([128, 128], F32, tag="ident")
            from concourse.masks import make_identity
            make_identity(nc, identity[:])

            def wqk(which, c):  # 0=q, 1=k
                return wqk_sb[:, (c * 2 + which) * P : (c * 2 + which + 1) * P]

            def wv(c):
                return wv_sb[:, c * P : (c + 1) * P]

            # Q^T [p, i] (fp32r single-pass)
            qt_ps = psA.tile([128, 128], F32, tag="qtps")
            nc.tensor.matmul(qt_ps[:], wqk(0, 0), xq_sb[:, 0:128], start=True, stop=False)
            nc.tensor.matmul(qt_ps[:], wqk(0, 1), xq_sb[:, 128:256], start=False, stop=True)
            qt_sb = cpool.tile([128, 128], F32R, tag="qt")
            nc.vector.tensor_scalar_add(qt_sb[:], qt_ps[:], bq_c)

            kt_t = [None] * 4
            v_t = [None] * 4
            stm_t = [None] * 4

            def make_tiles(jt: int):
                # K^T tile [p, s_jt]
                ktp = psB.tile([128, 128], F32, tag="proj", name=f"ktp{jt}")
                nc.tensor.matmul(ktp[:], wqk(1, 0), xt_t[jt][:, 0:128], start=True, stop=False)
                nc.tensor.matmul(ktp[:], wqk(1, 1), xt_t[jt][:, 128:256], start=False, stop=True)
                ktile = cpool.tile([128, 128], F32R, tag=f"kt{jt}", name=f"kt{jt}")
                nc.vector.tensor_scalar_add(ktile[:], ktp[:], bk_c)
                kt_t[jt] = ktile

                # scores [i, j_jt] -> sigmoid -> transpose -> mask
                sp = psB.tile([128, 128], F32, tag="sps", name=f"sps{jt}")
                nc.tensor.matmul(sp[:], qt_sb[:], ktile[:], start=True, stop=True)
                st = cpool.tile([128, 128], F32, tag=f"st{jt}", name=f"st{jt}")
                nc.scalar.activation(
                    st[:], sp[:], mybir.ActivationFunctionType.Sigmoid,
                    scale=INV_SQRT_P,
                )
                stp = psB.tile([128, 128], F32, tag="tp", name=f"stp{jt}")
                nc.tensor.transpose(stp[:], st[:], identity[:])
                stm = cpool.tile([128, 128], F32, tag=f"stm{jt}", name=f"stm{jt}")
                nc.vector.tensor_mul(stm[:], stp[:], mk_t[jt][:])
                stm_t[jt] = stm

                # V^T tile [p, s_jt] -> +bias -> transpose -> V [s_jt, p]
                vtp = psB.tile([128, 128], F32, tag="proj", name=f"vtp{jt}")
                nc.tensor.matmul(vtp[:], wv(0), xt_t[jt][:, 0:128], start=True, stop=False)
                nc.tensor.matmul(vtp[:], wv(1), xt_t[jt][:, 128:256], start=False, stop=True)
                vT = cpool.tile([128, 128], F32, tag=f"vT{jt}", name=f"vT{jt}")
                nc.vector.tensor_scalar_add(vT[:], vtp[:], bv_c)
                vp = psB.tile([128, 128], F32, tag="tp", name=f"vp{jt}")
                nc.tensor.transpose(vp[:], vT[:], identity[:])
                vt = cpool.tile([128, P], F32, tag=f"v{jt}", name=f"v{jt}")
                nc.vector.tensor_copy(vt[:], vp[:])
                v_t[jt] = vt

            # ---- output slab stage ----
            # groups of GROUP rows; class t = g//4 writes j-tiles 0..t.
            # slab free layout (jt, i, p); DMA [j_part, t, (i p)]: 4 KB runs.
            # Producers: greedy cost-balanced over DVE (batched TT), ACT
            # (per-row activation-scale), GpSimd (per-row tensor_scalar).
            out_r = out.rearrange("(t j) i p -> j t (i p)", j=128)  # [128,4,16384]
            busy = {"D": 0.0, "A": 0.0}
            cost = {"D": 1.25, "A": 3.9}

            def emit_group(g: int):
                L = g // (NGROUPS // 4) + 1
                slab = spool.tile(
                    [128, L * GROUP * 128], F32, tag=f"slab{L}", name=f"slab_g{g}"
                )
                for jt in range(L):
                    dst3 = slab[
                        :, jt * GROUP * 128 : (jt + 1) * GROUP * 128
                    ].rearrange("q (i p) -> q i p", i=GROUP)
                    eng = min(busy, key=lambda e: busy[e] + cost[e])
                    busy[eng] += cost[eng]
                    if eng == "D":
                        v3 = v_t[jt][:].unsqueeze(1).broadcast_to([128, GROUP, 128])
                        s3 = (
                            stm_t[jt][:, g * GROUP : (g + 1) * GROUP]
                            .unsqueeze(2)
                            .broadcast_to([128, GROUP, 128])
                        )
                        nc.vector.tensor_mul(dst3, v3, s3)
                    else:
                        for ii in range(GROUP):
                            li = g * GROUP + ii
                            if eng == "A":
                                nc.scalar.mul(
                                    dst3[:, ii, :],
                                    v_t[jt][:],
                                    mul=stm_t[jt][:, li : li + 1],
                                )
                            else:
                                nc.gpsimd.tensor_scalar_mul(
                                    dst3[:, ii, :],
                                    v_t[jt][:],
                                    stm_t[jt][:, li : li + 1],
                                )
                nc.sync.dma_start(
                    out_r[:, 0:L, GROUP * 128 * g : GROUP * 128 * (g + 1)],
                    slab[:].rearrange("q (t ip) -> q t ip", t=L),
                )

            make_tiles(0)
            for g in range(0, 4):      # class 0
                emit_group(g)
            for jt in range(1, 4):
                make_tiles(jt)
            for g in range(12, 16):    # class 3
                emit_group(g)
            for g in range(8, 12):     # class 2
                emit_group(g)
            for g in range(4, 8):      # class 1
                emit_group(g)

    _split_multi_waits(nc)
    return nc


def _split_multi_waits(nc):
    """This toolchain's walrus accepts at most one sync wait per
    instruction; split extras into single-wait NoOps just before the
    instruction on the same engine queue (waits are ANDed preconditions,
    executed in order on the engine's queue — semantically identical)."""
    for fn in nc.m.functions:
        for blk in fn.blocks:
            insts = blk.instructions
            i = 0
            while i < len(insts):
                inst = insts[i]
                si = getattr(inst, "sync_info", None)
                if si is not None and si.on_wait is not None and len(si.on_wait) > 1:
                    waits = list(si.on_wait)
                    nops = [
                        mybir.InstNoOp(
                            name=nc.get_next_instruction_name(),
                            engine=inst.engine,
                            sync_info=mybir.SyncInfo(on_wait=[w], on_update=[]),
                            bass_nofuse=True,
                        )
                        for w in waits[:-1]
                    ]
                    si.on_wait = [waits[-1]]
                    insts[i:i] = nops
                    i += len(nops)
                i += 1


_NC_CACHE = None


def _get_nc():
    global _NC_CACHE
    if _NC_CACHE is None:
        _NC_CACHE = _build_nc()
    return _NC_CACHE


def _in_maps(x_set, Wq, bq, Wk, bk, Wv, bv):
    w3 = np.ascontiguousarray(
        np.concatenate([Wq.T, Wk.T, Wv.T], axis=1)
    ).astype(np.float32, copy=False)
    b3 = np.ascontiguousarray(np.stack([bq, bk, bv], axis=1)).astype(np.float32, copy=False)
    xts = [
        np.ascontiguousarray(x_set[b].T).astype(np.float32, copy=False)
        for b in range(B)
    ]
    jj = np.arange(128)
    maps = []
    for c in range(NCORES):
        b, k = divmod(c, 4)
        rows = _rows_sel(k)
        mask = np.empty((4, 128, 128), np.float32)
        for jt in range(4):
            mask[jt] = ((jt * 128 + jj)[:, None] <= rows[None, :]).astype(np.float32)
        maps.append(
            {
                "xt": xts[b],
                "xq": np.ascontiguousarray(xts[b][:, rows]),
                "w3": w3,
                "b3": b3,
                "mk": mask,
            }
        )
    return maps


def run(x_set, Wq, bq, Wk, bk, Wv, bv, **spmd_kwargs):
    nc = _get_nc()
    in_maps = _in_maps(x_set, Wq, bq, Wk, bk, Wv, bv)
    res = bass_utils.run_bass_kernel_spmd(
        nc, in_maps, core_ids=list(range(NCORES)), **spmd_kwargs
    )
    full = np.zeros((B, S, S, P), np.float32)
    for c in range(NCORES):
        b, k = divmod(c, 4)
        # core output is [j, i_local, p] -> scatter as [i_local, j, p]
        full[b, _rows_sel(k)] = res.results[c]["out"].transpose(1, 0, 2)
    return full, res


def kernel(x_set, Wq, bq, Wk, bk, Wv, bv):
    full, _ = run(x_set, Wq, bq, Wk, bk, Wv, bv)
    return full
